# revision 2
# baseline (speedup 1.0000x reference)
"""Trainium2 Bass kernel for nn_CombinatorialClassifierSplit.

Reference computation:
    xr = x.reshape(B, P, S)
    logits = einsum('bps,pks', xr, W) + b          # (B, P, K)
    logp = log_softmax(logits, axis=2)
    out[b, c] = sum_p logp[b, p, idx[p, c]]        # (B, C)

Key restructuring: since idx doesn't depend on b,
    out[b, c] = sum_p logits[b, p, idx[p, c]] - LSE[b]
with LSE[b] = sum_p logsumexp_k(logits[b, p, :]).  The first term is a
plain matmul  x_flat @ Wg + bsum[c]  where Wg[(p,s), c] = W[p, idx[p,c], s]
and bsum[c] = sum_p b[p, idx[p,c]] are host-side gathers of the *static*
index tensor.  The device then runs, per core (classes C sharded 8 ways):
  - block-diagonal matmuls for logits -> exp -> segmented sum -> ln -> LSE
  - one big bf16 matmul (contract 2048) over its C-shard
  - + bsum via a rank-1 matmul, - LSE via activation bias, DMA out.
"""

import numpy as np
import ml_dtypes

import concourse.bacc as bacc
import concourse.tile as tile
from concourse import mybir
from concourse.bass_utils import run_bass_kernel_spmd

BF16 = ml_dtypes.bfloat16

B, P, K, S, C = 128, 32, 100, 64, 10000
N_CORES = 8
CS = C // N_CORES          # 1250 classes per core
NT = (P * S) // 128        # 16 contract chunks of 128
# c-tiles per core (PSUM bank is 512 fp32 wide)
C_TILES = [(0, 417), (417, 417), (834, 416)]

_cached = {}


def _build_program():
    if "nc" in _cached:
        return _cached["nc"]

    nc = bacc.Bacc("TRN2", target_bir_lowering=False, debug=False,
                   num_devices=N_CORES)
    dt = mybir.dt

    xt_d = nc.dram_tensor("xt", [128, NT, 128], dt.bfloat16, kind="ExternalInput")
    wg_d = nc.dram_tensor("wg", [128, NT, CS], dt.bfloat16, kind="ExternalInput")
    wk_d = nc.dram_tensor("wk", [128, NT, 200], dt.bfloat16, kind="ExternalInput")
    bias_d = nc.dram_tensor("bias", [1, P * K], dt.bfloat16, kind="ExternalInput")
    bsum_d = nc.dram_tensor("bsum", [1, CS], dt.bfloat16, kind="ExternalInput")
    ones_d = nc.dram_tensor("ones", [1, 128], dt.bfloat16, kind="ExternalInput")
    out_d = nc.dram_tensor("out", [128, CS], dt.float32, kind="ExternalOutput")

    with tile.TileContext(nc) as tc:
        with (
            tc.tile_pool(name="const", bufs=1) as cpool,
            tc.tile_pool(name="outp", bufs=3) as opool,
            tc.tile_pool(name="psum", bufs=8, space="PSUM") as ppool,
        ):
            xt_sb = cpool.tile([128, NT, 128], dt.bfloat16)
            wk_sb = cpool.tile([128, NT, 200], dt.bfloat16)
            bias_sb = cpool.tile([1, P * K], dt.bfloat16)
            bsum_sb = cpool.tile([1, CS], dt.bfloat16)
            ones_sb = cpool.tile([1, 128], dt.bfloat16)
            wg_sb = cpool.tile([128, NT, CS], dt.bfloat16)
            exp_sb = cpool.tile([128, P, K], dt.float32)
            sums_sb = cpool.tile([128, P], dt.float32)
            lns_sb = cpool.tile([128, P], dt.float32)
            nlse_sb = cpool.tile([128, 1], dt.float32)

            # --- input DMAs ---
            nc.sync.dma_start(xt_sb[:], xt_d[:])
            nc.sync.dma_start(wk_sb[:], wk_d[:])
            nc.sync.dma_start(bias_sb[:], bias_d[:])
            nc.sync.dma_start(bsum_sb[:], bsum_d[:])
            nc.sync.dma_start(ones_sb[:], ones_d[:])
            for t in range(NT):
                nc.sync.dma_start(wg_sb[:, t, :], wg_d[:, t, :])

            # --- logits -> exp (each psum tile holds 2 chunk-pairs = 4 p's) ---
            for j in range(NT // 2):
                ps = ppool.tile([128, 512], dt.float32, tag="ps")
                for h in range(2):
                    t = 2 * j + h
                    reg = ps[:, h * 200:(h + 1) * 200]
                    nc.tensor.matmul(reg, xt_sb[:, t, :], wk_sb[:, t, :],
                                     start=True, stop=False)
                    nc.tensor.matmul(reg, ones_sb[:],
                                     bias_sb[:, t * 200:(t + 1) * 200],
                                     start=False, stop=True)
                nc.scalar.activation(exp_sb[:, 4 * j:4 * j + 4, :], ps[:, 0:400],
                                     mybir.ActivationFunctionType.Exp)

            # --- LSE ---
            nc.vector.tensor_reduce(sums_sb[:], exp_sb[:],
                                    axis=mybir.AxisListType.X,
                                    op=mybir.AluOpType.add)
            nc.scalar.activation(lns_sb[:], sums_sb[:],
                                 mybir.ActivationFunctionType.Ln)
            nc.vector.tensor_reduce(nlse_sb[:], lns_sb[:],
                                    axis=mybir.AxisListType.X,
                                    op=mybir.AluOpType.add, negate=True)

            # --- main matmul over C-shard ---
            for (c0, cn) in C_TILES:
                ps = ppool.tile([128, 512], dt.float32, tag="ps")
                for i in range(NT):
                    nc.tensor.matmul(ps[:, 0:cn], xt_sb[:, i, :],
                                     wg_sb[:, i, c0:c0 + cn],
                                     start=(i == 0), stop=False)
                nc.tensor.matmul(ps[:, 0:cn], ones_sb[:],
                                 bsum_sb[:, c0:c0 + cn],
                                 start=False, stop=True)
                ot = opool.tile([128, 512], dt.float32, tag="ot")
                nc.scalar.activation(ot[:, 0:cn], ps[:, 0:cn],
                                     mybir.ActivationFunctionType.Identity,
                                     bias=nlse_sb[:])
                nc.sync.dma_start(out_d[:, c0:c0 + cn], ot[:, 0:cn])

    nc.compile()
    _cached["nc"] = nc
    return nc


def _prep_inputs(x, W, b, idx):
    """Host-side data prep -> per-core input maps."""
    x = np.asarray(x, dtype=np.float32)
    W = np.asarray(W, dtype=np.float32)
    b = np.asarray(b, dtype=np.float32)
    idx = np.asarray(idx, dtype=np.int64)

    # x^T in (s_local, chunk, b) layout
    xt = np.ascontiguousarray(
        x.T.reshape(NT, 128, B).transpose(1, 0, 2)).astype(BF16)

    # block-diagonal per-pair weights for the logits path: (128, NT, 200)
    wk = np.zeros((128, NT, 200), dtype=np.float32)
    for t in range(NT):
        wk[0:64, t, 0:100] = W[2 * t].T       # (s, k)
        wk[64:128, t, 100:200] = W[2 * t + 1].T
    wk = wk.astype(BF16)

    bias = np.ascontiguousarray(b.reshape(1, P * K)).astype(BF16)
    ones = np.ones((1, 128), dtype=BF16)

    # gathered big weight matrix: Wg[(p,s), c] = W[p, idx[p,c], s]
    Wg = W[np.arange(P)[:, None], idx]            # (P, C, S)
    Wg = np.ascontiguousarray(Wg.transpose(0, 2, 1)).reshape(P * S, C)
    bsum_full = b[np.arange(P)[:, None], idx].sum(axis=0)   # (C,)

    in_maps = []
    for m in range(N_CORES):
        sl = Wg[:, m * CS:(m + 1) * CS]
        wg = np.ascontiguousarray(
            sl.reshape(NT, 128, CS).transpose(1, 0, 2)).astype(BF16)
        bsum = bsum_full[m * CS:(m + 1) * CS].reshape(1, CS).astype(BF16)
        in_maps.append({
            "xt": xt, "wg": wg, "wk": wk, "bias": bias,
            "bsum": bsum, "ones": ones,
        })
    return in_maps


def kernel(x, W, b, partitionings):
    nc = _build_program()
    in_maps = _prep_inputs(x, W, b, partitionings)
    res = run_bass_kernel_spmd(nc, in_maps, list(range(N_CORES)))
    out = np.concatenate([np.asarray(res.results[m]["out"])
                          for m in range(N_CORES)], axis=1)
    return out.astype(np.float32)


# revision 13
# speedup vs baseline: 1.1272x; 1.1272x over previous
"""Trainium2 Bass kernel for nn_CombinatorialClassifierSplit.

Reference computation:
    xr = x.reshape(B, P, S)
    logits = einsum('bps,pks', xr, W) + b          # (B, P, K)
    logp = log_softmax(logits, axis=2)
    out[b, c] = sum_p logp[b, p, idx[p, c]]        # (B, C)

Key restructuring: since idx doesn't depend on b,
    out[b, c] = sum_p logits[b, p, idx[p, c]] - LSE[b]
with LSE[b] = sum_p logsumexp_k(logits[b, p, :]).  The first term is a
plain matmul  x_flat @ Wg + bsum[c]  where Wg[(p,s), c] = W[p, idx[p,c], s]
and bsum[c] = sum_p b[p, idx[p,c]] are host-side gathers of the *static*
index tensor.  The device then runs, per core (classes C sharded 8 ways):
  - per-p matmuls for logits -> exp -> segmented sum -> ln -> -LSE
  - one big bf16 matmul (contract 2048) over its C-shard, c-tile by c-tile
  - + bsum via a rank-1 matmul, - LSE via DVE scalar add, DMA out.
"""

import numpy as np
import ml_dtypes

import concourse.bacc as bacc
import concourse.tile as tile
from concourse import mybir
from concourse.bass_utils import run_bass_kernel_spmd

BF16 = ml_dtypes.bfloat16

B, P, K, S, C = 128, 32, 100, 64, 10000
N_CORES = 8
CS = C // N_CORES          # 1250 classes per core
NT = (P * S) // 128        # 16 contract chunks of 128
# c-tiles per core (PSUM bank is 512 fp32 wide); last tile kept small so
# the dependent tail (last wg piece -> matmul -> add -> out DMA) is short
C_TILES = [(0, 480), (480, 480), (960, 290)]
# aux tensor layout: [bias (P*K) | bsum (CS) | ones (128)]
AUX_BIAS, AUX_BSUM, AUX_ONES = 0, P * K, P * K + CS
AUX_LEN = P * K + CS + 128

_cached = {}


def _build_program():
    if "nc" in _cached:
        return _cached["nc"]

    nc = bacc.Bacc("TRN2", target_bir_lowering=False, debug=False,
                   num_devices=N_CORES)
    dt = mybir.dt

    xt_d = nc.dram_tensor("xt", [128, NT, 128], dt.bfloat16, kind="ExternalInput")
    wg_d = nc.dram_tensor("wg", [128, NT, CS], dt.bfloat16, kind="ExternalInput")
    wk_d = nc.dram_tensor("wk", [128, NT, K], dt.bfloat16, kind="ExternalInput")
    aux_d = nc.dram_tensor("aux", [1, AUX_LEN], dt.bfloat16, kind="ExternalInput")
    out_d = nc.dram_tensor("out", [128, CS], dt.float32, kind="ExternalOutput")

    with tile.TileContext(nc) as tc:
        with (
            tc.tile_pool(name="const", bufs=1) as cpool,
            tc.tile_pool(name="outp", bufs=3) as opool,
            tc.tile_pool(name="psum", bufs=8, space="PSUM") as ppool,
        ):
            xt_sb = cpool.tile([128, NT, 128], dt.bfloat16)
            wk_sb = cpool.tile([128, NT, K], dt.bfloat16)
            aux_sb = cpool.tile([1, AUX_LEN], dt.bfloat16)
            wg_sb = cpool.tile([128, NT, CS], dt.bfloat16)
            exp_sb = cpool.tile([128, P, K], dt.bfloat16)
            sums_sb = cpool.tile([128, P], dt.float32)
            lns_sb = cpool.tile([128, P], dt.float32)
            nlse_sb = cpool.tile([128, 1], dt.float32)

            bias = lambda lo, n: aux_sb[:, AUX_BIAS + lo:AUX_BIAS + lo + n]
            bsum = lambda lo, n: aux_sb[:, AUX_BSUM + lo:AUX_BSUM + lo + n]
            ones_ap = aux_sb[:, AUX_ONES:AUX_ONES + 128]

            # --- input DMAs (first xt chunks + wk + aux unblock the LSE
            # chain early; wg tiles stream after, tail tile sub-split so the
            # last matmuls overlap the final DMA pieces) ---
            nc.sync.dma_start(wk_sb[:], wk_d[:])
            nc.sync.dma_start(xt_sb[:], xt_d[:])
            nc.sync.dma_start(aux_sb[:], aux_d[:])
            WG_SPLITS = [[(0, 16)], [(0, 8), (8, 16)],
                         [(0, 8), (8, 12), (12, 15), (15, 16)]]
            for (c0, cn), splits in zip(C_TILES, WG_SPLITS):
                for (i0, i1) in splits:
                    nc.sync.dma_start(wg_sb[:, i0:i1, c0:c0 + cn],
                                      wg_d[:, i0:i1, c0:c0 + cn])

            # --- logits -> exp (each psum tile holds 4 p's) ---
            for j in range(P // 4):
                ps = ppool.tile([128, 512], dt.float32, tag="ps")
                for q in range(4):
                    p = 4 * j + q
                    t, h = p // 2, p % 2
                    reg = ps[:, q * K:(q + 1) * K]
                    nc.tensor.matmul(reg, ones_ap, bias(p * K, K),
                                     start=True, stop=False)
                    nc.tensor.matmul(reg,
                                     xt_sb[h * 64:h * 64 + 64, t, :],
                                     wk_sb[h * 64:h * 64 + 64, t, :],
                                     start=False, stop=True)
                nc.scalar.activation(exp_sb[:, 4 * j:4 * j + 4, :],
                                     ps[:, 0:4 * K],
                                     mybir.ActivationFunctionType.Exp)
                nc.vector.tensor_reduce(sums_sb[:, 4 * j:4 * j + 4],
                                        exp_sb[:, 4 * j:4 * j + 4, :],
                                        axis=mybir.AxisListType.X,
                                        op=mybir.AluOpType.add)

            # --- LSE ---
            nc.scalar.activation(lns_sb[:], sums_sb[:],
                                 mybir.ActivationFunctionType.Ln)
            nc.vector.tensor_reduce(nlse_sb[:], lns_sb[:],
                                    axis=mybir.AxisListType.X,
                                    op=mybir.AluOpType.add, negate=True)

            # --- main matmul over C-shard, c-tile outer ---
            ADD_ENGINE = "dve"
            FINAL_SPLIT = 1
            for ti, (c0, cn) in enumerate(C_TILES):
                ps = ppool.tile([128, 512], dt.float32, tag="ps")
                nc.tensor.matmul(ps[:, 0:cn], ones_ap, bsum(c0, cn),
                                 start=True, stop=False)
                for i in range(NT):
                    nc.tensor.matmul(ps[:, 0:cn], xt_sb[:, i, :],
                                     wg_sb[:, i, c0:c0 + cn],
                                     start=False, stop=(i == NT - 1))
                ot = opool.tile([128, 512], dt.float32, tag="ot")
                nsp = FINAL_SPLIT if ti == len(C_TILES) - 1 else 1
                bounds = [(cn * s // nsp, cn * (s + 1) // nsp - cn * s // nsp)
                          for s in range(nsp)]
                for (h0, hn) in bounds:
                    if ADD_ENGINE == "act":
                        nc.scalar.activation(
                            ot[:, h0:h0 + hn], ps[:, h0:h0 + hn],
                            mybir.ActivationFunctionType.Identity,
                            bias=nlse_sb[:])
                    else:
                        nc.vector.tensor_scalar_add(ot[:, h0:h0 + hn],
                                                    ps[:, h0:h0 + hn],
                                                    nlse_sb[:])
                    nc.sync.dma_start(out_d[:, c0 + h0:c0 + h0 + hn],
                                      ot[:, h0:h0 + hn])

    nc.compile()
    _cached["nc"] = nc
    return nc


def _prep_inputs(x, W, b, idx):
    """Host-side data prep -> per-core input maps."""
    x = np.asarray(x, dtype=np.float32)
    W = np.asarray(W, dtype=np.float32)
    b = np.asarray(b, dtype=np.float32)
    idx = np.asarray(idx, dtype=np.int64)

    # x^T in (s_local, chunk, b) layout
    xt = np.ascontiguousarray(
        x.T.reshape(NT, 128, B).transpose(1, 0, 2)).astype(BF16)

    # packed per-pair weights for the logits path: (128, NT, K)
    # rows [0:64, t] = W[2t].T ; rows [64:128, t] = W[2t+1].T
    wk = np.empty((128, NT, K), dtype=np.float32)
    for t in range(NT):
        wk[0:64, t, :] = W[2 * t].T
        wk[64:128, t, :] = W[2 * t + 1].T
    wk = wk.astype(BF16)

    # gathered big weight matrix: Wg[(p,s), c] = W[p, idx[p,c], s]
    Wg = W[np.arange(P)[:, None], idx]            # (P, C, S)
    Wg = np.ascontiguousarray(Wg.transpose(0, 2, 1)).reshape(P * S, C)
    bsum_full = b[np.arange(P)[:, None], idx].sum(axis=0)   # (C,)

    aux_base = np.zeros((1, AUX_LEN), dtype=np.float32)
    aux_base[0, AUX_BIAS:AUX_BIAS + P * K] = b.reshape(-1)
    aux_base[0, AUX_ONES:AUX_ONES + 128] = 1.0

    in_maps = []
    for m in range(N_CORES):
        sl = Wg[:, m * CS:(m + 1) * CS]
        wg = np.ascontiguousarray(
            sl.reshape(NT, 128, CS).transpose(1, 0, 2)).astype(BF16)
        aux = aux_base.copy()
        aux[0, AUX_BSUM:AUX_BSUM + CS] = bsum_full[m * CS:(m + 1) * CS]
        in_maps.append({"xt": xt, "wg": wg, "wk": wk,
                        "aux": aux.astype(BF16)})
    return in_maps


def kernel(x, W, b, partitionings):
    nc = _build_program()
    in_maps = _prep_inputs(x, W, b, partitionings)
    res = run_bass_kernel_spmd(nc, in_maps, list(range(N_CORES)))
    out = np.concatenate([np.asarray(res.results[m]["out"])
                          for m in range(N_CORES)], axis=1)
    return out.astype(np.float32)
